# revision 27
# baseline (speedup 1.0000x reference)
"""Segment-mean + projection kernel for Trainium2 (8 NeuronCores, SPMD).

logits[b] = (mean of x rows in bag b) @ rel_weight.T + bias

Strategy (v3): data-parallel over bags. Bags are split by size:
  - big bags (count >= 4): rows quantized to fp8 e4m3, segment-summed with
    DoubleRow matmuls (K=256 rows per PE pass, 2 fp8 MACs/cell/cycle).
  - small bags (count <= 3): rows in fp8 e3m4 (4 mantissa bits), normal
    matmuls. Small bags dominate the max quantization error (a 1-row bag's
    mean is the row itself), so they get the extra mantissa bit.
Groups are bag-aligned (a bag never spans groups -> no fixup pass):
big groups hold up to 1024 rows / 128 bags (4 DoubleRow pair-tiles),
small groups up to 128 bags / 384 rows (3 tiles). Per group the PE
accumulates one-hot.T @ x into PSUM [128 bags, 690], ScalarE copies out
with 1/count scale to fp16 means, PE transposes 6 chunks of 128, DVE/
ScalarE stage them into a [128, 6*256] tile per group pair, and the PE
projects against W.T chunks (fp16), bias added, emitted as logitsT
[53, 256] per pair; the host compacts valid columns.

A post-pass drops duplicate back-to-back LDWEIGHTS (the two matmuls of a
PSUM pair share one stationary one-hot), halving PE weight-load time.
"""
import sys
import re

sys.path.insert(0, "/opt/trn_rl_repo")

import numpy as np
import ml_dtypes

N_CORES = 8
D = 690
D_SPLIT = 344
C = 53
D_CHUNKS = 6

BIG_ROWS = 1024      # rows per big group (4 DoubleRow pair-tiles)
BIG_SETS = 8         # row-sets of 128 (pair-tile pt, half ko) per big group
SMALL_TILES = 3      # tiles of 128 rows per small group
SMALL_ROWS = 128 * SMALL_TILES
MAX_BAGS = 128       # bag slots per group (PSUM partition dim)
SMALL_MAX = 3        # bag size threshold: <= SMALL_MAX goes to e3m4 region

E4 = ml_dtypes.float8_e4m3
E3 = ml_dtypes.float8_e3m4


def _apply_walrus_workarounds():
    """This walrus build allows at most one semaphore wait per instruction
    on several opcodes (Drain, Matmult/LDW). Patch Tile's tail drain to use
    standalone wait_ge instructions, and provide a post-pass that hoists
    excess waits onto InstNoOp instructions."""
    from concourse import tile, mybir

    def _patched_drain_and_barrier(self, tick_clock, wait_clock):
        gc = tick_clock.global_clock
        ticks = [int(s) for s in re.findall(r"\d+", repr(gc))]
        allocated = self.sems.allocated()
        for proc, sem in sorted(allocated.items()):
            t = ticks[proc] if proc < len(ticks) else 0
            if t > 0:
                mult = 16 if "DMA" in sem.name else 1
                self.nc.sync.wait_ge(sem, t * mult)
        self.nc.sync.drain()
        self.nc.all_engine_barrier()
        popped = self.nc._tile_sem_poison_stack.pop()
        assert popped is self._sem_poison
        self.nc.clear_and_free_semaphores(list(allocated.values()))
        self.nc.all_engine_barrier()

    tile.TileContext._drain_and_barrier = _patched_drain_and_barrier

    def split_multi_waits(nc, max_waits=1):
        for f in nc.m.functions:
            for b in f.blocks:
                insts = list(b.instructions)
                new = []
                dirty = False
                for inst in insts:
                    si = inst.sync_info
                    if si is not None and len(si.on_wait) > max_waits:
                        waits = list(si.on_wait)
                        extra, keep = waits[:-max_waits], waits[-max_waits:]
                        for k, w in enumerate(extra):
                            nop = mybir.InstNoOp(
                                name=f"{inst.name}-hw{k}", ins=[], outs=[]
                            )
                            nop.engine = inst.engine
                            nop.sync_info = mybir.SyncInfo(
                                on_wait=[w], on_update=[]
                            )
                            new.append(nop)
                        inst.sync_info = mybir.SyncInfo(
                            on_wait=keep, on_update=list(si.on_update)
                        )
                        dirty = True
                    new.append(inst)
                if dirty:
                    b.instructions = new

    return split_multi_waits


def _dedup_ldweights(nc):
    """Drop an InstLdweights whose weights AP is byte-identical to the
    immediately preceding PE weight load (no other PE weight load between).
    The paired matmuls then reuse the already-loaded stationary. Waits and
    semaphore updates of a dropped load are preserved on a PE InstNoOp."""
    from concourse import mybir

    n_dropped = 0
    for f in nc.m.functions:
        for b in f.blocks:
            insts = list(b.instructions)
            new = []
            last_sig = None
            dirty = False
            for inst in insts:
                if isinstance(inst, mybir.InstLdweights):
                    sig = (
                        repr(inst.ins[0]),
                        getattr(inst, "perf_mode", None),
                        getattr(inst, "is_transpose", None),
                    )
                    if sig == last_sig:
                        si = inst.sync_info
                        if si is not None and (si.on_wait or si.on_update):
                            nop = mybir.InstNoOp(
                                name=f"{inst.name}-dd", ins=[], outs=[]
                            )
                            nop.engine = inst.engine
                            nop.sync_info = mybir.SyncInfo(
                                on_wait=list(si.on_wait),
                                on_update=list(si.on_update),
                            )
                            new.append(nop)
                        n_dropped += 1
                        dirty = True
                        continue
                    last_sig = sig
                new.append(inst)
            if dirty:
                b.instructions = new
    return n_dropped


def _pack_groups(bag_ids, counts, max_rows, max_bags):
    """Greedy bag-aligned packing: consecutive bags into groups obeying
    row and bag-slot limits. Returns list of lists of bag ids."""
    groups = []
    cur = []
    cur_rows = 0
    for b in bag_ids:
        n = int(counts[b])
        if cur and (cur_rows + n > max_rows or len(cur) >= max_bags):
            groups.append(cur)
            cur = []
            cur_rows = 0
        cur.append(b)
        cur_rows += n
    if cur:
        groups.append(cur)
    return groups


def _preprocess(x, scope, n_cores=N_CORES):
    n_sent = x.shape[0]
    n_bags = scope.shape[0] - 1
    scope = np.asarray(scope, dtype=np.int64)
    counts = np.diff(scope)
    assert counts.min() >= 1
    assert counts.max() <= BIG_ROWS

    # bag-aligned core cuts near k * n_sent / n_cores
    bag_cuts = [0]
    for k in range(1, n_cores):
        t = (k * n_sent) // n_cores
        b = int(np.searchsorted(scope, t, side="right")) - 1
        bag_cuts.append(b)
    bag_cuts.append(n_bags)

    x_e4 = np.vstack([x.astype(E4), np.zeros((1, D), dtype=E4)])
    x_e3 = np.vstack([x.astype(E3), np.zeros((1, D), dtype=E3)])

    per_core = []
    for c in range(n_cores):
        b0, b1 = bag_cuts[c], bag_cuts[c + 1]
        ids = np.arange(b0, b1)
        big = ids[counts[ids] > SMALL_MAX]
        small = ids[counts[ids] <= SMALL_MAX]
        bgroups = _pack_groups(big, counts, BIG_ROWS, MAX_BAGS)
        sgroups = _pack_groups(small, counts, SMALL_ROWS, MAX_BAGS)
        per_core.append((bgroups, sgroups))

    GB = max(len(pc[0]) for pc in per_core)
    GS = max(len(pc[1]) for pc in per_core)
    if (GB + GS) % 2:
        GS += 1
    G = GB + GS
    n_pairs = G // 2

    cores = []
    for c in range(n_cores):
        bgroups, sgroups = per_core[c]
        # row-source index per slot; n_sent = zero row sentinel
        idx_big = np.full((GB, BIG_ROWS), n_sent, dtype=np.int64)
        idx_small = np.full((GS, SMALL_ROWS), n_sent, dtype=np.int64)
        seg_big = np.full((GB, BIG_ROWS), 128.0, dtype=np.float32)
        seg_small = np.full((GS, SMALL_ROWS), 128.0, dtype=np.float32)
        recip = np.ones((G, 128), dtype=np.float32)
        meta = []  # per group: global bag ids (np array)

        for g, bags in enumerate(bgroups):
            pos = 0
            for m, b in enumerate(bags):
                n = int(counts[b])
                idx_big[g, pos : pos + n] = np.arange(scope[b], scope[b + 1])
                seg_big[g, pos : pos + n] = m
                recip[g, m] = 1.0 / n
                pos += n
        for g, bags in enumerate(sgroups):
            pos = 0
            for m, b in enumerate(bags):
                n = int(counts[b])
                idx_small[g, pos : pos + n] = np.arange(scope[b], scope[b + 1])
                seg_small[g, pos : pos + n] = m
                recip[GB + g, m] = 1.0 / n
                pos += n
        for g in range(G):
            if g < GB:
                bags = bgroups[g] if g < len(bgroups) else []
            else:
                bags = sgroups[g - GB] if g - GB < len(sgroups) else []
            meta.append(np.asarray(bags, dtype=np.int64))

        # big region: slot s -> (set j = (s//256)*2 + (s%256)//128, ki = s%128)
        # DRAM layout [128, GB*8*690]: partition ki, free (g*8 + j)*690 + d
        xb = x_e4[idx_big.reshape(GB, 4, 2, 128)]        # [GB,pt,ko,ki,D]
        xb = np.ascontiguousarray(
            xb.reshape(GB * 8, 128, D).transpose(1, 0, 2)
        ).reshape(128, GB * 8 * D)
        sb = np.ascontiguousarray(
            seg_big.reshape(GB * 8, 128).T
        )                                                # [128, GB*8]

        xs = x_e3[idx_small.reshape(GS, SMALL_TILES, 128)]
        xs = np.ascontiguousarray(
            xs.reshape(GS * SMALL_TILES, 128, D).transpose(1, 0, 2)
        ).reshape(128, GS * SMALL_TILES * D)
        ss = np.ascontiguousarray(seg_small.reshape(GS * SMALL_TILES, 128).T)

        cores.append(
            dict(
                x_big=xb,
                x_small=xs,
                seg_big=sb,
                seg_small=ss,
                recip=np.ascontiguousarray(recip.T),     # [128, G]
                meta=meta,
            )
        )
    return cores, GB, GS, G, n_pairs


def _build_program(GB, GS, G, n_pairs, serial=False):
    import concourse.bass as bass
    import concourse.mybir as mybir
    from concourse import tile

    dt = mybir.dt
    nc = bass.Bass()
    DR = mybir.MatmulPerfMode.DoubleRow

    x_big_d = nc.declare_dram_parameter(
        "x_big", [128, GB * 8 * D], dt.float8e4, isOutput=False
    )
    x_small_d = nc.declare_dram_parameter(
        "x_small", [128, GS * SMALL_TILES * D], dt.float8e3, isOutput=False
    )
    seg_big_d = nc.declare_dram_parameter(
        "seg_big", [128, GB * 8], dt.float32, isOutput=False
    )
    seg_small_d = nc.declare_dram_parameter(
        "seg_small", [128, GS * SMALL_TILES], dt.float32, isOutput=False
    )
    recip_d = nc.declare_dram_parameter("recip", [128, G], dt.float32, isOutput=False)
    iota_d = nc.declare_dram_parameter("iota", [128, 128], dt.float32, isOutput=False)
    ident_d = nc.declare_dram_parameter("ident", [128, 128], dt.float16, isOutput=False)
    wt_d = nc.declare_dram_parameter("wt", [128, 768], dt.float16, isOutput=False)
    bias_d = nc.declare_dram_parameter("bias", [C, 1], dt.float32, isOutput=False)
    out_d = nc.declare_dram_parameter(
        "out", [C, n_pairs * 256], dt.float32, isOutput=True
    )

    B = (lambda n: 1) if serial else (lambda n: n)

    with tile.TileContext(nc) as tc:
        with (
            tc.tile_pool(name="const", bufs=1) as cpool,
            tc.tile_pool(name="xb", bufs=B(4)) as xbpool,
            tc.tile_pool(name="xs", bufs=B(3)) as xspool,
            tc.tile_pool(name="onehot", bufs=B(8)) as apool,
            tc.tile_pool(name="means", bufs=B(2)) as mpool,
            tc.tile_pool(name="mgt", bufs=B(2)) as tpool,
            tc.tile_pool(name="outs", bufs=B(2)) as opool,
            tc.tile_pool(name="ps_sum", bufs=B(2), space="PSUM") as pspool,
            tc.tile_pool(name="ps_tr", bufs=B(2), space="PSUM") as ptpool,
            tc.tile_pool(name="ps_proj", bufs=B(2), space="PSUM") as pppool,
        ):
            iota_t = cpool.tile([128, 128], dt.float32)
            ident_t = cpool.tile([128, 128], dt.float16)
            seg_b_t = cpool.tile([128, GB * 8], dt.float32)
            seg_s_t = cpool.tile([128, GS * SMALL_TILES], dt.float32)
            recip_t = cpool.tile([128, G], dt.float32)
            wt_t = cpool.tile([128, 768], dt.float16)
            bias_t = cpool.tile([C, 1], dt.float32)

            nc.gpsimd.dma_start(out=iota_t[:], in_=iota_d[:])
            nc.gpsimd.dma_start(out=ident_t[:], in_=ident_d[:])
            nc.gpsimd.dma_start(out=seg_b_t[:], in_=seg_big_d[:])
            nc.gpsimd.dma_start(out=seg_s_t[:], in_=seg_small_d[:])
            nc.gpsimd.dma_start(out=recip_t[:], in_=recip_d[:])
            nc.gpsimd.dma_start(out=wt_t[:], in_=wt_d[:])
            nc.gpsimd.dma_start(out=bias_t[:], in_=bias_d[:])

            # warm the PE HAM while the first x batches are in flight
            ps_w = ptpool.tile([128, 2, 128], dt.float16, tag="pt")
            for _ in range(48):
                nc.tensor.transpose(ps_w[:, 0, :], ident_t[:], ident_t[:])

            mgt = None
            xb3 = None
            xs2 = None
            for g in range(G):
                big = g < GB
                ps_a = pspool.tile([128, D_SPLIT], dt.float32, tag="psa")
                ps_b = pspool.tile([128, D - D_SPLIT], dt.float32, tag="psb")

                if big:
                    xb = xbpool.tile([128, 8, D], dt.float8e4, tag="xb")
                    nc.sync.dma_start(
                        out=xb[:], in_=x_big_d[:, g * 8 * D : (g + 1) * 8 * D]
                    )
                    for pt in range(4):
                        a_t = apool.tile([128, 2, 128], dt.float8e4, tag="a")
                        for ko in range(2):
                            col = g * 8 + pt * 2 + ko
                            nc.vector.tensor_scalar(
                                out=a_t[:, ko, :],
                                in0=iota_t[:],
                                scalar1=seg_b_t[:, col : col + 1],
                                scalar2=None,
                                op0=mybir.AluOpType.is_equal,
                            )
                        first = pt == 0
                        last = pt == 3
                        j0 = pt * 2
                        nc.tensor.matmul(
                            ps_a[:],
                            a_t[:],
                            xb[:, j0 : j0 + 2, 0:D_SPLIT],
                            start=first,
                            stop=last,
                            perf_mode=DR,
                        )
                        nc.tensor.matmul(
                            ps_b[:],
                            a_t[:],
                            xb[:, j0 : j0 + 2, D_SPLIT:D],
                            start=first,
                            stop=last,
                            perf_mode=DR,
                        )
                else:
                    gs = g - GB
                    xs = xspool.tile([128, SMALL_TILES, D], dt.float8e3, tag="xs")
                    nc.sync.dma_start(
                        out=xs[:],
                        in_=x_small_d[
                            :, gs * SMALL_TILES * D : (gs + 1) * SMALL_TILES * D
                        ],
                    )
                    for t in range(SMALL_TILES):
                        a_s = apool.tile([128, 128], dt.float8e3, tag="a")
                        col = gs * SMALL_TILES + t
                        nc.vector.tensor_scalar(
                            out=a_s[:],
                            in0=iota_t[:],
                            scalar1=seg_s_t[:, col : col + 1],
                            scalar2=None,
                            op0=mybir.AluOpType.is_equal,
                        )
                        first = t == 0
                        last = t == SMALL_TILES - 1
                        nc.tensor.matmul(
                            ps_a[:],
                            a_s[:],
                            xs[:, t, 0:D_SPLIT],
                            start=first,
                            stop=last,
                        )
                        nc.tensor.matmul(
                            ps_b[:],
                            a_s[:],
                            xs[:, t, D_SPLIT:D],
                            start=first,
                            stop=last,
                        )

                # means = psum * (1/count), fp16, padded to 768 cols
                means = mpool.tile([128, 768], dt.float16, tag="m")
                nc.scalar.activation(
                    means[:, 0:D_SPLIT],
                    ps_a[:],
                    mybir.ActivationFunctionType.Copy,
                    scale=recip_t[:, g : g + 1],
                )
                nc.scalar.activation(
                    means[:, D_SPLIT:D],
                    ps_b[:],
                    mybir.ActivationFunctionType.Copy,
                    scale=recip_t[:, g : g + 1],
                )
                if g < 2:
                    # first use of each double-buffered means tile; the pad
                    # columns are never written again, zeros persist
                    nc.vector.memset(means[:, D:768], 0.0)

                h = g % 2
                if h == 0:
                    mgt = tpool.tile([128, 6, 256], dt.float16, tag="mgt")
                for d in range(D_CHUNKS):
                    ps_t = ptpool.tile([128, 2, 128], dt.float16, tag="pt")
                    nc.tensor.transpose(
                        ps_t[:, 0, :],
                        means[:, d * 128 : (d + 1) * 128],
                        ident_t[:],
                    )
                    dst = mgt[:, d, h * 128 : h * 128 + 128]
                    if d % 2 == 0:
                        nc.vector.tensor_copy(dst, ps_t[:, 0, :])
                    else:
                        nc.scalar.copy(dst, ps_t[:, 0, :])
                if h == 1:
                    q = g // 2
                    pp = pppool.tile([128, 256], dt.float32, tag="pp")
                    for d in range(D_CHUNKS):
                        nc.tensor.matmul(
                            pp[:],
                            wt_t[:, d * 128 : (d + 1) * 128],
                            mgt[:, d, :],
                            start=(d == 0),
                            stop=(d == D_CHUNKS - 1),
                        )
                    out_sb = opool.tile([C, 256], dt.float32, tag="o")
                    nc.scalar.activation(
                        out_sb[:],
                        pp[0:C, :],
                        mybir.ActivationFunctionType.Identity,
                        bias=bias_t[:],
                    )
                    nc.sync.dma_start(
                        out=out_d[:, q * 256 : (q + 1) * 256], in_=out_sb[:]
                    )
    return nc


def prepare(x, scope, rel_weight, bias, serial=False, dedup=True):
    split_multi_waits = _apply_walrus_workarounds()

    x = np.asarray(x, dtype=np.float32)
    scope_np = np.asarray(scope)
    rel_weight = np.asarray(rel_weight, dtype=np.float32)
    bias = np.asarray(bias, dtype=np.float32)
    n_bags = scope_np.shape[0] - 1

    cores, GB, GS, G, n_pairs = _preprocess(x, scope_np)
    nc = _build_program(GB, GS, G, n_pairs, serial=serial)
    if dedup:
        _dedup_ldweights(nc)
    split_multi_waits(nc)

    iota = np.tile(np.arange(128, dtype=np.float32), (128, 1))
    ident = np.eye(128, dtype=np.float16)
    wt = np.zeros((128, 768), dtype=np.float16)
    wpad = np.zeros((C, 768), dtype=np.float32)
    wpad[:, :D] = rel_weight
    for d in range(6):
        wt[:, d * 128 : d * 128 + C] = wpad[:, d * 128 : (d + 1) * 128].T
    bias_in = bias.reshape(C, 1).copy()

    in_maps = []
    for c in range(N_CORES):
        cd = cores[c]
        in_maps.append(
            {
                "x_big": cd["x_big"],
                "x_small": cd["x_small"],
                "seg_big": cd["seg_big"],
                "seg_small": cd["seg_small"],
                "recip": cd["recip"],
                "iota": iota,
                "ident": ident,
                "wt": wt,
                "bias": bias_in,
            }
        )

    def assemble(results):
        logits_t = np.empty((C, n_bags), dtype=np.float32)
        for c in range(N_CORES):
            out = results[c]["out"]  # [C, n_pairs*256]
            meta = cores[c]["meta"]
            for g in range(G):
                bags = meta[g]
                if len(bags) == 0:
                    continue
                col0 = 256 * (g // 2) + 128 * (g % 2)
                logits_t[:, bags] = out[:, col0 : col0 + len(bags)]
        return np.ascontiguousarray(logits_t.T)

    return dict(
        nc=nc, in_maps=in_maps, assemble=assemble, G=G, GB=GB, GS=GS,
        n_pairs=n_pairs,
    )


def kernel(x, scope, rel_weight, bias):
    from concourse.bass_utils import run_bass_kernel_spmd

    p = prepare(x, scope, rel_weight, bias)
    res = run_bass_kernel_spmd(p["nc"], p["in_maps"], list(range(N_CORES)))
    return p["assemble"](res.results)


# revision 28
# speedup vs baseline: 1.0216x; 1.0216x over previous
"""Segment-mean + projection kernel for Trainium2 (8 NeuronCores, SPMD).

logits[b] = (mean of x rows in bag b) @ rel_weight.T + bias

Strategy (v3): data-parallel over bags. Bags are split by size:
  - big bags (count >= 4): rows quantized to fp8 e4m3, segment-summed with
    DoubleRow matmuls (K=256 rows per PE pass, 2 fp8 MACs/cell/cycle).
  - small bags (count <= 3): rows in fp8 e3m4 (4 mantissa bits), normal
    matmuls. Small bags dominate the max quantization error (a 1-row bag's
    mean is the row itself), so they get the extra mantissa bit.
Groups are bag-aligned (a bag never spans groups -> no fixup pass):
big groups hold up to 1024 rows / 128 bags (4 DoubleRow pair-tiles),
small groups up to 128 bags / 384 rows (3 tiles). Per group the PE
accumulates one-hot.T @ x into PSUM [128 bags, 690], ScalarE copies out
with 1/count scale to fp16 means, PE transposes 6 chunks of 128, DVE/
ScalarE stage them into a [128, 6*256] tile per group pair, and the PE
projects against W.T chunks (fp16), bias added, emitted as logitsT
[53, 256] per pair; the host compacts valid columns.

A post-pass drops duplicate back-to-back LDWEIGHTS (the two matmuls of a
PSUM pair share one stationary one-hot), halving PE weight-load time.
"""
import sys
import re

sys.path.insert(0, "/opt/trn_rl_repo")

import numpy as np
import ml_dtypes

N_CORES = 8
D = 690
D_SPLIT = 344
C = 53
D_CHUNKS = 6

BIG_ROWS = 1024      # rows per big group (4 DoubleRow pair-tiles)
BIG_SETS = 8         # row-sets of 128 (pair-tile pt, half ko) per big group
SMALL_TILES = 3      # tiles of 128 rows per small group
SMALL_ROWS = 128 * SMALL_TILES
MAX_BAGS = 128       # bag slots per group (PSUM partition dim)
SMALL_MAX = 3        # bag size threshold: <= SMALL_MAX goes to e3m4 region

E4 = ml_dtypes.float8_e4m3
E3 = ml_dtypes.float8_e3m4


def _apply_walrus_workarounds():
    """This walrus build allows at most one semaphore wait per instruction
    on several opcodes (Drain, Matmult/LDW). Patch Tile's tail drain to use
    standalone wait_ge instructions, and provide a post-pass that hoists
    excess waits onto InstNoOp instructions."""
    from concourse import tile, mybir

    def _patched_drain_and_barrier(self, tick_clock, wait_clock):
        gc = tick_clock.global_clock
        ticks = [int(s) for s in re.findall(r"\d+", repr(gc))]
        allocated = self.sems.allocated()
        for proc, sem in sorted(allocated.items()):
            t = ticks[proc] if proc < len(ticks) else 0
            if t > 0:
                mult = 16 if "DMA" in sem.name else 1
                self.nc.sync.wait_ge(sem, t * mult)
        self.nc.sync.drain()
        self.nc.all_engine_barrier()
        popped = self.nc._tile_sem_poison_stack.pop()
        assert popped is self._sem_poison
        self.nc.clear_and_free_semaphores(list(allocated.values()))
        self.nc.all_engine_barrier()

    tile.TileContext._drain_and_barrier = _patched_drain_and_barrier

    def split_multi_waits(nc, max_waits=1):
        for f in nc.m.functions:
            for b in f.blocks:
                insts = list(b.instructions)
                new = []
                dirty = False
                for inst in insts:
                    si = inst.sync_info
                    if si is not None and len(si.on_wait) > max_waits:
                        waits = list(si.on_wait)
                        extra, keep = waits[:-max_waits], waits[-max_waits:]
                        for k, w in enumerate(extra):
                            nop = mybir.InstNoOp(
                                name=f"{inst.name}-hw{k}", ins=[], outs=[]
                            )
                            nop.engine = inst.engine
                            nop.sync_info = mybir.SyncInfo(
                                on_wait=[w], on_update=[]
                            )
                            new.append(nop)
                        inst.sync_info = mybir.SyncInfo(
                            on_wait=keep, on_update=list(si.on_update)
                        )
                        dirty = True
                    new.append(inst)
                if dirty:
                    b.instructions = new

    return split_multi_waits


def _dedup_ldweights(nc):
    """Drop an InstLdweights whose weights AP is byte-identical to the
    immediately preceding PE weight load (no other PE weight load between).
    The paired matmuls then reuse the already-loaded stationary. Waits and
    semaphore updates of a dropped load are preserved on a PE InstNoOp."""
    from concourse import mybir

    n_dropped = 0
    for f in nc.m.functions:
        for b in f.blocks:
            insts = list(b.instructions)
            new = []
            last_sig = None
            dirty = False
            for inst in insts:
                if isinstance(inst, mybir.InstLdweights):
                    sig = (
                        repr(inst.ins[0]),
                        getattr(inst, "perf_mode", None),
                        getattr(inst, "is_transpose", None),
                    )
                    if sig == last_sig:
                        si = inst.sync_info
                        if si is not None and (si.on_wait or si.on_update):
                            nop = mybir.InstNoOp(
                                name=f"{inst.name}-dd", ins=[], outs=[]
                            )
                            nop.engine = inst.engine
                            nop.sync_info = mybir.SyncInfo(
                                on_wait=list(si.on_wait),
                                on_update=list(si.on_update),
                            )
                            new.append(nop)
                        n_dropped += 1
                        dirty = True
                        continue
                    last_sig = sig
                new.append(inst)
            if dirty:
                b.instructions = new
    return n_dropped


def _pack_groups(bag_ids, counts, max_rows, max_bags):
    """Greedy bag-aligned packing: consecutive bags into groups obeying
    row and bag-slot limits. Returns list of lists of bag ids."""
    groups = []
    cur = []
    cur_rows = 0
    for b in bag_ids:
        n = int(counts[b])
        if cur and (cur_rows + n > max_rows or len(cur) >= max_bags):
            groups.append(cur)
            cur = []
            cur_rows = 0
        cur.append(b)
        cur_rows += n
    if cur:
        groups.append(cur)
    return groups


def _preprocess(x, scope, n_cores=N_CORES):
    n_sent = x.shape[0]
    n_bags = scope.shape[0] - 1
    scope = np.asarray(scope, dtype=np.int64)
    counts = np.diff(scope)
    assert counts.min() >= 1
    assert counts.max() <= BIG_ROWS

    # bag-aligned core cuts near k * n_sent / n_cores
    bag_cuts = [0]
    for k in range(1, n_cores):
        t = (k * n_sent) // n_cores
        b = int(np.searchsorted(scope, t, side="right")) - 1
        bag_cuts.append(b)
    bag_cuts.append(n_bags)

    x_e4 = np.vstack([x.astype(E4), np.zeros((1, D), dtype=E4)])
    x_e3 = np.vstack([x.astype(E3), np.zeros((1, D), dtype=E3)])

    per_core = []
    for c in range(n_cores):
        b0, b1 = bag_cuts[c], bag_cuts[c + 1]
        ids = np.arange(b0, b1)
        big = ids[counts[ids] > SMALL_MAX]
        small = ids[counts[ids] <= SMALL_MAX]
        bgroups = _pack_groups(big, counts, BIG_ROWS, MAX_BAGS)
        sgroups = _pack_groups(small, counts, SMALL_ROWS, MAX_BAGS)
        per_core.append((bgroups, sgroups))

    GB = max(len(pc[0]) for pc in per_core)
    GS = max(len(pc[1]) for pc in per_core)
    if (GB + GS) % 2:
        GS += 1
    G = GB + GS
    n_pairs = G // 2

    cores = []
    for c in range(n_cores):
        bgroups, sgroups = per_core[c]
        # row-source index per slot; n_sent = zero row sentinel
        idx_big = np.full((GB, BIG_ROWS), n_sent, dtype=np.int64)
        idx_small = np.full((GS, SMALL_ROWS), n_sent, dtype=np.int64)
        seg_big = np.full((GB, BIG_ROWS), 128.0, dtype=np.float32)
        seg_small = np.full((GS, SMALL_ROWS), 128.0, dtype=np.float32)
        recip = np.ones((G, 128), dtype=np.float32)
        meta = []  # per group: global bag ids (np array)

        for g, bags in enumerate(bgroups):
            pos = 0
            for m, b in enumerate(bags):
                n = int(counts[b])
                idx_big[g, pos : pos + n] = np.arange(scope[b], scope[b + 1])
                seg_big[g, pos : pos + n] = m
                recip[g, m] = 1.0 / n
                pos += n
        for g, bags in enumerate(sgroups):
            pos = 0
            for m, b in enumerate(bags):
                n = int(counts[b])
                idx_small[g, pos : pos + n] = np.arange(scope[b], scope[b + 1])
                seg_small[g, pos : pos + n] = m
                recip[GB + g, m] = 1.0 / n
                pos += n
        for g in range(G):
            if g < GB:
                bags = bgroups[g] if g < len(bgroups) else []
            else:
                bags = sgroups[g - GB] if g - GB < len(sgroups) else []
            meta.append(np.asarray(bags, dtype=np.int64))

        # big region: slot s -> (set j = (s//256)*2 + (s%256)//128, ki = s%128)
        # DRAM layout [128, GB*8*690]: partition ki, free (g*8 + j)*690 + d
        xb = x_e4[idx_big.reshape(GB, 4, 2, 128)]        # [GB,pt,ko,ki,D]
        xb = np.ascontiguousarray(
            xb.reshape(GB * 8, 128, D).transpose(1, 0, 2)
        ).reshape(128, GB * 8 * D)
        sb = np.ascontiguousarray(
            seg_big.reshape(GB * 8, 128).T
        )                                                # [128, GB*8]

        xs = x_e3[idx_small.reshape(GS, SMALL_TILES, 128)]
        xs = np.ascontiguousarray(
            xs.reshape(GS * SMALL_TILES, 128, D).transpose(1, 0, 2)
        ).reshape(128, GS * SMALL_TILES * D)
        ss = np.ascontiguousarray(seg_small.reshape(GS * SMALL_TILES, 128).T)

        cores.append(
            dict(
                x_big=xb,
                x_small=xs,
                seg_big=sb,
                seg_small=ss,
                recip=np.ascontiguousarray(recip.T),     # [128, G]
                meta=meta,
            )
        )
    return cores, GB, GS, G, n_pairs


def _build_program(GB, GS, G, n_pairs, serial=False):
    import concourse.bass as bass
    import concourse.mybir as mybir
    from concourse import tile

    dt = mybir.dt
    nc = bass.Bass()
    DR = mybir.MatmulPerfMode.DoubleRow

    x_big_d = nc.declare_dram_parameter(
        "x_big", [128, GB * 8 * D], dt.float8e4, isOutput=False
    )
    x_small_d = nc.declare_dram_parameter(
        "x_small", [128, GS * SMALL_TILES * D], dt.float8e3, isOutput=False
    )
    seg_big_d = nc.declare_dram_parameter(
        "seg_big", [128, GB * 8], dt.float32, isOutput=False
    )
    seg_small_d = nc.declare_dram_parameter(
        "seg_small", [128, GS * SMALL_TILES], dt.float32, isOutput=False
    )
    recip_d = nc.declare_dram_parameter("recip", [128, G], dt.float32, isOutput=False)
    iota_d = nc.declare_dram_parameter("iota", [128, 128], dt.float32, isOutput=False)
    ident_d = nc.declare_dram_parameter("ident", [128, 128], dt.float16, isOutput=False)
    wt_d = nc.declare_dram_parameter("wt", [128, 768], dt.float16, isOutput=False)
    bias_d = nc.declare_dram_parameter("bias", [C, 1], dt.float32, isOutput=False)
    out_d = nc.declare_dram_parameter(
        "out", [C, n_pairs * 256], dt.float32, isOutput=True
    )

    B = (lambda n: 1) if serial else (lambda n: n)

    with tile.TileContext(nc) as tc:
        with (
            tc.tile_pool(name="const", bufs=1) as cpool,
            tc.tile_pool(name="xb", bufs=B(4)) as xbpool,
            tc.tile_pool(name="xs", bufs=B(3)) as xspool,
            tc.tile_pool(name="onehot", bufs=B(8)) as apool,
            tc.tile_pool(name="means", bufs=B(2)) as mpool,
            tc.tile_pool(name="mgt", bufs=B(2)) as tpool,
            tc.tile_pool(name="outs", bufs=B(2)) as opool,
            tc.tile_pool(name="ps_sum", bufs=B(2), space="PSUM") as pspool,
            tc.tile_pool(name="ps_tr", bufs=B(2), space="PSUM") as ptpool,
            tc.tile_pool(name="ps_proj", bufs=B(2), space="PSUM") as pppool,
        ):
            iota_t = cpool.tile([128, 128], dt.float32)
            ident_t = cpool.tile([128, 128], dt.float16)
            seg_b_t = cpool.tile([128, GB * 8], dt.float32)
            seg_s_t = cpool.tile([128, GS * SMALL_TILES], dt.float32)
            recip_t = cpool.tile([128, G], dt.float32)
            wt_t = cpool.tile([128, 768], dt.float16)
            bias_t = cpool.tile([C, 1], dt.float32)

            nc.gpsimd.dma_start(out=iota_t[:], in_=iota_d[:])
            nc.gpsimd.dma_start(out=ident_t[:], in_=ident_d[:])
            nc.gpsimd.dma_start(out=seg_b_t[:], in_=seg_big_d[:])
            nc.gpsimd.dma_start(out=seg_s_t[:], in_=seg_small_d[:])
            nc.gpsimd.dma_start(out=recip_t[:], in_=recip_d[:])
            nc.gpsimd.dma_start(out=wt_t[:], in_=wt_d[:])
            nc.gpsimd.dma_start(out=bias_t[:], in_=bias_d[:])

            # warm the PE HAM while the first x batches are in flight
            ps_w = ptpool.tile([128, 2, 128], dt.float16, tag="pt")
            for _ in range(48):
                nc.tensor.transpose(ps_w[:, 0, :], ident_t[:], ident_t[:])

            mgt = None
            xb3 = None
            xs2 = None
            for g in range(G):
                big = g < GB
                ps_a = pspool.tile([128, D_SPLIT], dt.float32, tag="psa")
                ps_b = pspool.tile([128, D - D_SPLIT], dt.float32, tag="psb")

                if big:
                    xb = xbpool.tile([128, 8, D], dt.float8e4, tag="xb")
                    nc.sync.dma_start(
                        out=xb[:], in_=x_big_d[:, g * 8 * D : (g + 1) * 8 * D]
                    )
                    for pt in range(4):
                        a_t = apool.tile([128, 2, 128], dt.float8e4, tag="a")
                        for ko in range(2):
                            col = g * 8 + pt * 2 + ko
                            nc.vector.tensor_scalar(
                                out=a_t[:, ko, :],
                                in0=iota_t[:],
                                scalar1=seg_b_t[:, col : col + 1],
                                scalar2=None,
                                op0=mybir.AluOpType.is_equal,
                            )
                        first = pt == 0
                        last = pt == 3
                        j0 = pt * 2
                        nc.tensor.matmul(
                            ps_a[:],
                            a_t[:],
                            xb[:, j0 : j0 + 2, 0:D_SPLIT],
                            start=first,
                            stop=last,
                            perf_mode=DR,
                        )
                        nc.tensor.matmul(
                            ps_b[:],
                            a_t[:],
                            xb[:, j0 : j0 + 2, D_SPLIT:D],
                            start=first,
                            stop=last,
                            perf_mode=DR,
                        )
                else:
                    gs = g - GB
                    xs = xspool.tile([128, SMALL_TILES, D], dt.float8e3, tag="xs")
                    nc.sync.dma_start(
                        out=xs[:],
                        in_=x_small_d[
                            :, gs * SMALL_TILES * D : (gs + 1) * SMALL_TILES * D
                        ],
                    )
                    for t in range(SMALL_TILES):
                        a_s = apool.tile([128, 128], dt.float8e3, tag="a")
                        col = gs * SMALL_TILES + t
                        nc.vector.tensor_scalar(
                            out=a_s[:],
                            in0=iota_t[:],
                            scalar1=seg_s_t[:, col : col + 1],
                            scalar2=None,
                            op0=mybir.AluOpType.is_equal,
                        )
                        first = t == 0
                        last = t == SMALL_TILES - 1
                        nc.tensor.matmul(
                            ps_a[:],
                            a_s[:],
                            xs[:, t, 0:D_SPLIT],
                            start=first,
                            stop=last,
                        )
                        nc.tensor.matmul(
                            ps_b[:],
                            a_s[:],
                            xs[:, t, D_SPLIT:D],
                            start=first,
                            stop=last,
                        )

                # means = psum * (1/count), fp16, padded to 768 cols
                means = mpool.tile([128, 768], dt.float16, tag="m")
                nc.scalar.activation(
                    means[:, 0:D_SPLIT],
                    ps_a[:],
                    mybir.ActivationFunctionType.Copy,
                    scale=recip_t[:, g : g + 1],
                )
                nc.scalar.activation(
                    means[:, D_SPLIT:D],
                    ps_b[:],
                    mybir.ActivationFunctionType.Copy,
                    scale=recip_t[:, g : g + 1],
                )
                if g < 2:
                    # first use of each double-buffered means tile; the pad
                    # columns are never written again, zeros persist
                    nc.vector.memset(means[:, D:768], 0.0)

                h = g % 2
                if h == 0:
                    mgt = tpool.tile([128, 6, 256], dt.float16, tag="mgt")
                for d in range(D_CHUNKS):
                    ps_t = ptpool.tile([128, 2, 128], dt.float16, tag="pt")
                    nc.tensor.transpose(
                        ps_t[:, 0, :],
                        means[:, d * 128 : (d + 1) * 128],
                        ident_t[:],
                    )
                    dst = mgt[:, d, h * 128 : h * 128 + 128]
                    if d % 2 == 0:
                        nc.vector.tensor_copy(dst, ps_t[:, 0, :])
                    else:
                        nc.scalar.copy(dst, ps_t[:, 0, :])
                if h == 1:
                    q = g // 2
                    pp = pppool.tile([128, 256], dt.float32, tag="pp")
                    for d in range(D_CHUNKS):
                        nc.tensor.matmul(
                            pp[:],
                            wt_t[:, d * 128 : (d + 1) * 128],
                            mgt[:, d, :],
                            start=(d == 0),
                            stop=(d == D_CHUNKS - 1),
                        )
                    out_sb = opool.tile([C, 256], dt.float32, tag="o")
                    nc.scalar.activation(
                        out_sb[:],
                        pp[0:C, :],
                        mybir.ActivationFunctionType.Identity,
                        bias=bias_t[:],
                    )
                    nc.scalar.dma_start(
                        out=out_d[:, q * 256 : (q + 1) * 256], in_=out_sb[:]
                    )
    return nc


def prepare(x, scope, rel_weight, bias, serial=False, dedup=True):
    split_multi_waits = _apply_walrus_workarounds()

    x = np.asarray(x, dtype=np.float32)
    scope_np = np.asarray(scope)
    rel_weight = np.asarray(rel_weight, dtype=np.float32)
    bias = np.asarray(bias, dtype=np.float32)
    n_bags = scope_np.shape[0] - 1

    cores, GB, GS, G, n_pairs = _preprocess(x, scope_np)
    nc = _build_program(GB, GS, G, n_pairs, serial=serial)
    if dedup:
        _dedup_ldweights(nc)
    split_multi_waits(nc)

    iota = np.tile(np.arange(128, dtype=np.float32), (128, 1))
    ident = np.eye(128, dtype=np.float16)
    wt = np.zeros((128, 768), dtype=np.float16)
    wpad = np.zeros((C, 768), dtype=np.float32)
    wpad[:, :D] = rel_weight
    for d in range(6):
        wt[:, d * 128 : d * 128 + C] = wpad[:, d * 128 : (d + 1) * 128].T
    bias_in = bias.reshape(C, 1).copy()

    in_maps = []
    for c in range(N_CORES):
        cd = cores[c]
        in_maps.append(
            {
                "x_big": cd["x_big"],
                "x_small": cd["x_small"],
                "seg_big": cd["seg_big"],
                "seg_small": cd["seg_small"],
                "recip": cd["recip"],
                "iota": iota,
                "ident": ident,
                "wt": wt,
                "bias": bias_in,
            }
        )

    def assemble(results):
        logits_t = np.empty((C, n_bags), dtype=np.float32)
        for c in range(N_CORES):
            out = results[c]["out"]  # [C, n_pairs*256]
            meta = cores[c]["meta"]
            for g in range(G):
                bags = meta[g]
                if len(bags) == 0:
                    continue
                col0 = 256 * (g // 2) + 128 * (g % 2)
                logits_t[:, bags] = out[:, col0 : col0 + len(bags)]
        return np.ascontiguousarray(logits_t.T)

    return dict(
        nc=nc, in_maps=in_maps, assemble=assemble, G=G, GB=GB, GS=GS,
        n_pairs=n_pairs,
    )


def kernel(x, scope, rel_weight, bias):
    from concourse.bass_utils import run_bass_kernel_spmd

    p = prepare(x, scope, rel_weight, bias)
    res = run_bass_kernel_spmd(p["nc"], p["in_maps"], list(range(N_CORES)))
    return p["assemble"](res.results)


# revision 29
# speedup vs baseline: 1.1463x; 1.1221x over previous
"""Segment-mean + projection kernel for Trainium2 (8 NeuronCores, SPMD).

logits[b] = (mean of x rows in bag b) @ rel_weight.T + bias

Strategy (v3): data-parallel over bags. Bags are split by size:
  - big bags (count >= 4): rows quantized to fp8 e4m3, segment-summed with
    DoubleRow matmuls (K=256 rows per PE pass, 2 fp8 MACs/cell/cycle).
  - small bags (count <= 3): rows in fp8 e3m4 (4 mantissa bits), normal
    matmuls. Small bags dominate the max quantization error (a 1-row bag's
    mean is the row itself), so they get the extra mantissa bit.
Groups are bag-aligned (a bag never spans groups -> no fixup pass):
big groups hold up to 1024 rows / 128 bags (4 DoubleRow pair-tiles),
small groups up to 128 bags / 384 rows (3 tiles). Per group the PE
accumulates one-hot.T @ x into PSUM [128 bags, 690], ScalarE copies out
with 1/count scale to fp16 means, PE transposes 6 chunks of 128, DVE/
ScalarE stage them into a [128, 6*256] tile per group pair, and the PE
projects against W.T chunks (fp16), bias added, emitted as logitsT
[53, 256] per pair; the host compacts valid columns.

A post-pass drops duplicate back-to-back LDWEIGHTS (the two matmuls of a
PSUM pair share one stationary one-hot), halving PE weight-load time.
"""
import sys
import re

sys.path.insert(0, "/opt/trn_rl_repo")

import numpy as np
import ml_dtypes

N_CORES = 8
D = 690
D_SPLIT = 344
C = 53
D_CHUNKS = 6

BIG_ROWS = 1024      # rows per big group (4 DoubleRow pair-tiles)
BIG_SETS = 8         # row-sets of 128 (pair-tile pt, half ko) per big group
SMALL_TILES = 3      # tiles of 128 rows per small group
SMALL_ROWS = 128 * SMALL_TILES
MAX_BAGS = 128       # bag slots per group (PSUM partition dim)
SMALL_MAX = 3        # bag size threshold: <= SMALL_MAX goes to e3m4 region

E4 = ml_dtypes.float8_e4m3
E3 = ml_dtypes.float8_e3m4


def _apply_walrus_workarounds():
    """This walrus build allows at most one semaphore wait per instruction
    on several opcodes (Drain, Matmult/LDW). Patch Tile's tail drain to use
    standalone wait_ge instructions, and provide a post-pass that hoists
    excess waits onto InstNoOp instructions."""
    from concourse import tile, mybir

    def _patched_drain_and_barrier(self, tick_clock, wait_clock):
        gc = tick_clock.global_clock
        ticks = [int(s) for s in re.findall(r"\d+", repr(gc))]
        allocated = self.sems.allocated()
        for proc, sem in sorted(allocated.items()):
            t = ticks[proc] if proc < len(ticks) else 0
            if t > 0:
                mult = 16 if "DMA" in sem.name else 1
                self.nc.sync.wait_ge(sem, t * mult)
        self.nc.sync.drain()
        self.nc.all_engine_barrier()
        popped = self.nc._tile_sem_poison_stack.pop()
        assert popped is self._sem_poison
        self.nc.clear_and_free_semaphores(list(allocated.values()))
        self.nc.all_engine_barrier()

    tile.TileContext._drain_and_barrier = _patched_drain_and_barrier

    def split_multi_waits(nc, max_waits=1):
        for f in nc.m.functions:
            for b in f.blocks:
                insts = list(b.instructions)
                new = []
                dirty = False
                for inst in insts:
                    si = inst.sync_info
                    if si is not None and len(si.on_wait) > max_waits:
                        waits = list(si.on_wait)
                        extra, keep = waits[:-max_waits], waits[-max_waits:]
                        for k, w in enumerate(extra):
                            nop = mybir.InstNoOp(
                                name=f"{inst.name}-hw{k}", ins=[], outs=[]
                            )
                            nop.engine = inst.engine
                            nop.sync_info = mybir.SyncInfo(
                                on_wait=[w], on_update=[]
                            )
                            new.append(nop)
                        inst.sync_info = mybir.SyncInfo(
                            on_wait=keep, on_update=list(si.on_update)
                        )
                        dirty = True
                    new.append(inst)
                if dirty:
                    b.instructions = new

    return split_multi_waits


def _dedup_ldweights(nc):
    """Drop an InstLdweights whose weights AP is byte-identical to the
    immediately preceding PE weight load (no other PE weight load between).
    The paired matmuls then reuse the already-loaded stationary. Waits and
    semaphore updates of a dropped load are preserved on a PE InstNoOp."""
    from concourse import mybir

    n_dropped = 0
    for f in nc.m.functions:
        for b in f.blocks:
            insts = list(b.instructions)
            new = []
            last_sig = None
            dirty = False
            for inst in insts:
                if isinstance(inst, mybir.InstLdweights):
                    sig = (
                        repr(inst.ins[0]),
                        getattr(inst, "perf_mode", None),
                        getattr(inst, "is_transpose", None),
                    )
                    if sig == last_sig:
                        si = inst.sync_info
                        if si is not None and (si.on_wait or si.on_update):
                            nop = mybir.InstNoOp(
                                name=f"{inst.name}-dd", ins=[], outs=[]
                            )
                            nop.engine = inst.engine
                            nop.sync_info = mybir.SyncInfo(
                                on_wait=list(si.on_wait),
                                on_update=list(si.on_update),
                            )
                            new.append(nop)
                        n_dropped += 1
                        dirty = True
                        continue
                    last_sig = sig
                new.append(inst)
            if dirty:
                b.instructions = new
    return n_dropped


def _pack_groups(bag_ids, counts, max_rows, max_bags):
    """Greedy bag-aligned packing: consecutive bags into groups obeying
    row and bag-slot limits. Returns list of lists of bag ids."""
    groups = []
    cur = []
    cur_rows = 0
    for b in bag_ids:
        n = int(counts[b])
        if cur and (cur_rows + n > max_rows or len(cur) >= max_bags):
            groups.append(cur)
            cur = []
            cur_rows = 0
        cur.append(b)
        cur_rows += n
    if cur:
        groups.append(cur)
    return groups


def _preprocess(x, scope, n_cores=N_CORES):
    n_sent = x.shape[0]
    n_bags = scope.shape[0] - 1
    scope = np.asarray(scope, dtype=np.int64)
    counts = np.diff(scope)
    assert counts.min() >= 1
    assert counts.max() <= BIG_ROWS

    # bag-aligned core cuts near k * n_sent / n_cores
    bag_cuts = [0]
    for k in range(1, n_cores):
        t = (k * n_sent) // n_cores
        b = int(np.searchsorted(scope, t, side="right")) - 1
        bag_cuts.append(b)
    bag_cuts.append(n_bags)

    x_e4 = np.vstack([x.astype(E4), np.zeros((1, D), dtype=E4)])
    x_e3 = np.vstack([x.astype(E3), np.zeros((1, D), dtype=E3)])

    per_core = []
    for c in range(n_cores):
        b0, b1 = bag_cuts[c], bag_cuts[c + 1]
        ids = np.arange(b0, b1)
        big = ids[counts[ids] > SMALL_MAX]
        small = ids[counts[ids] <= SMALL_MAX]
        bgroups = _pack_groups(big, counts, BIG_ROWS, MAX_BAGS)
        sgroups = _pack_groups(small, counts, SMALL_ROWS, MAX_BAGS)
        per_core.append((bgroups, sgroups))

    GB = max(len(pc[0]) for pc in per_core)
    GS = max(len(pc[1]) for pc in per_core)
    if (GB + GS) % 2:
        GS += 1
    G = GB + GS
    n_pairs = G // 2

    cores = []
    for c in range(n_cores):
        bgroups, sgroups = per_core[c]
        # row-source index per slot; n_sent = zero row sentinel
        idx_big = np.full((GB, BIG_ROWS), n_sent, dtype=np.int64)
        idx_small = np.full((GS, SMALL_ROWS), n_sent, dtype=np.int64)
        seg_big = np.full((GB, BIG_ROWS), 128.0, dtype=np.float32)
        seg_small = np.full((GS, SMALL_ROWS), 128.0, dtype=np.float32)
        recip = np.ones((G, 128), dtype=np.float32)
        meta = []  # per group: global bag ids (np array)

        for g, bags in enumerate(bgroups):
            pos = 0
            for m, b in enumerate(bags):
                n = int(counts[b])
                idx_big[g, pos : pos + n] = np.arange(scope[b], scope[b + 1])
                seg_big[g, pos : pos + n] = m
                recip[g, m] = 1.0 / n
                pos += n
        for g, bags in enumerate(sgroups):
            pos = 0
            for m, b in enumerate(bags):
                n = int(counts[b])
                idx_small[g, pos : pos + n] = np.arange(scope[b], scope[b + 1])
                seg_small[g, pos : pos + n] = m
                recip[GB + g, m] = 1.0 / n
                pos += n
        for g in range(G):
            if g < GB:
                bags = bgroups[g] if g < len(bgroups) else []
            else:
                bags = sgroups[g - GB] if g - GB < len(sgroups) else []
            meta.append(np.asarray(bags, dtype=np.int64))

        # big region: slot s -> (set j = (s//256)*2 + (s%256)//128, ki = s%128)
        # DRAM layout [128, GB*8*690]: partition ki, free (g*8 + j)*690 + d
        xb = x_e4[idx_big.reshape(GB, 4, 2, 128)]        # [GB,pt,ko,ki,D]
        xb = np.ascontiguousarray(
            xb.reshape(GB * 8, 128, D).transpose(1, 0, 2)
        ).reshape(128, GB * 8 * D)
        sb = np.ascontiguousarray(
            seg_big.reshape(GB * 8, 128).T
        )                                                # [128, GB*8]

        xs = x_e3[idx_small.reshape(GS, SMALL_TILES, 128)]
        xs = np.ascontiguousarray(
            xs.reshape(GS * SMALL_TILES, 128, D).transpose(1, 0, 2)
        ).reshape(128, GS * SMALL_TILES * D)
        ss = np.ascontiguousarray(seg_small.reshape(GS * SMALL_TILES, 128).T)

        cores.append(
            dict(
                x_big=xb,
                x_small=xs,
                seg_big=sb,
                seg_small=ss,
                recip=np.ascontiguousarray(recip.T),     # [128, G]
                meta=meta,
            )
        )
    return cores, GB, GS, G, n_pairs


def _build_program(GB, GS, G, n_pairs, serial=False):
    import concourse.bass as bass
    import concourse.mybir as mybir
    from concourse import tile

    dt = mybir.dt
    nc = bass.Bass()
    DR = mybir.MatmulPerfMode.DoubleRow

    x_big_d = nc.declare_dram_parameter(
        "x_big", [128, GB * 8 * D], dt.float8e4, isOutput=False
    )
    x_small_d = nc.declare_dram_parameter(
        "x_small", [128, GS * SMALL_TILES * D], dt.float8e3, isOutput=False
    )
    seg_big_d = nc.declare_dram_parameter(
        "seg_big", [128, GB * 8], dt.float32, isOutput=False
    )
    seg_small_d = nc.declare_dram_parameter(
        "seg_small", [128, GS * SMALL_TILES], dt.float32, isOutput=False
    )
    recip_d = nc.declare_dram_parameter("recip", [128, G], dt.float32, isOutput=False)
    iota_d = nc.declare_dram_parameter("iota", [128, 128], dt.float32, isOutput=False)
    ident_d = nc.declare_dram_parameter("ident", [128, 128], dt.float16, isOutput=False)
    wt_d = nc.declare_dram_parameter("wt", [128, 768], dt.float16, isOutput=False)
    bias_d = nc.declare_dram_parameter("bias", [C, 1], dt.float32, isOutput=False)
    out_d = nc.declare_dram_parameter(
        "out", [C, n_pairs * 256], dt.float32, isOutput=True
    )

    B = (lambda n: 1) if serial else (lambda n: n)

    with tile.TileContext(nc) as tc:
        with (
            tc.tile_pool(name="const", bufs=1) as cpool,
            tc.tile_pool(name="xb", bufs=B(4)) as xbpool,
            tc.tile_pool(name="xs", bufs=B(3)) as xspool,
            tc.tile_pool(name="onehot", bufs=B(8)) as apool,
            tc.tile_pool(name="means", bufs=B(2)) as mpool,
            tc.tile_pool(name="mgt", bufs=B(2)) as tpool,
            tc.tile_pool(name="outs", bufs=B(2)) as opool,
            tc.tile_pool(name="ps_sum", bufs=B(2), space="PSUM") as pspool,
            tc.tile_pool(name="ps_tr", bufs=B(2), space="PSUM") as ptpool,
            tc.tile_pool(name="ps_proj", bufs=B(2), space="PSUM") as pppool,
        ):
            iota_t = cpool.tile([128, 128], dt.float32)
            ident_t = cpool.tile([128, 128], dt.float16)
            seg_b_t = cpool.tile([128, GB * 8], dt.float32)
            seg_s_t = cpool.tile([128, GS * SMALL_TILES], dt.float32)
            recip_t = cpool.tile([128, G], dt.float32)
            wt_t = cpool.tile([128, 768], dt.float16)
            bias_t = cpool.tile([C, 1], dt.float32)

            nc.gpsimd.dma_start(out=iota_t[:], in_=iota_d[:])
            nc.gpsimd.dma_start(out=ident_t[:], in_=ident_d[:])
            nc.gpsimd.dma_start(out=seg_b_t[:], in_=seg_big_d[:])
            nc.gpsimd.dma_start(out=seg_s_t[:], in_=seg_small_d[:])
            nc.gpsimd.dma_start(out=recip_t[:], in_=recip_d[:])
            nc.gpsimd.dma_start(out=wt_t[:], in_=wt_d[:])
            nc.gpsimd.dma_start(out=bias_t[:], in_=bias_d[:])

            # warm the PE HAM while the first x batches are in flight
            ps_w = ptpool.tile([128, 2, 128], dt.float16, tag="pt")
            for _ in range(48):
                nc.tensor.transpose(ps_w[:, 0, :], ident_t[:], ident_t[:])

            mgt = None
            xb3 = None
            xs2 = None
            for g in range(G):
                big = g < GB
                ps_a = pspool.tile([128, D_SPLIT], dt.float32, tag="psa")
                ps_b = pspool.tile([128, D - D_SPLIT], dt.float32, tag="psb")

                if big:
                    xb = xbpool.tile([128, 8, D], dt.float8e4, tag="xb")
                    nc.sync.dma_start(
                        out=xb[:], in_=x_big_d[:, g * 8 * D : (g + 1) * 8 * D]
                    )
                    for pt in range(4):
                        a_t = apool.tile([128, 2, 128], dt.float8e4, tag="a")
                        for ko in range(2):
                            col = g * 8 + pt * 2 + ko
                            nc.vector.tensor_scalar(
                                out=a_t[:, ko, :],
                                in0=iota_t[:],
                                scalar1=seg_b_t[:, col : col + 1],
                                scalar2=None,
                                op0=mybir.AluOpType.is_equal,
                            )
                        first = pt == 0
                        last = pt == 3
                        j0 = pt * 2
                        nc.tensor.matmul(
                            ps_a[:],
                            a_t[:],
                            xb[:, j0 : j0 + 2, 0:D_SPLIT],
                            start=first,
                            stop=last,
                            perf_mode=DR,
                        )
                        nc.tensor.matmul(
                            ps_b[:],
                            a_t[:],
                            xb[:, j0 : j0 + 2, D_SPLIT:D],
                            start=first,
                            stop=last,
                            perf_mode=DR,
                        )
                else:
                    gs = g - GB
                    xs = xspool.tile([128, SMALL_TILES, D], dt.float8e3, tag="xs")
                    nc.sync.dma_start(
                        out=xs[:],
                        in_=x_small_d[
                            :, gs * SMALL_TILES * D : (gs + 1) * SMALL_TILES * D
                        ],
                    )
                    for t in range(SMALL_TILES):
                        a_s = apool.tile([128, 128], dt.float8e3, tag="a")
                        col = gs * SMALL_TILES + t
                        nc.vector.tensor_scalar(
                            out=a_s[:],
                            in0=iota_t[:],
                            scalar1=seg_s_t[:, col : col + 1],
                            scalar2=None,
                            op0=mybir.AluOpType.is_equal,
                        )
                        first = t == 0
                        last = t == SMALL_TILES - 1
                        nc.tensor.matmul(
                            ps_a[:],
                            a_s[:],
                            xs[:, t, 0:D_SPLIT],
                            start=first,
                            stop=last,
                        )
                        nc.tensor.matmul(
                            ps_b[:],
                            a_s[:],
                            xs[:, t, D_SPLIT:D],
                            start=first,
                            stop=last,
                        )

                # means = psum * (1/count), fp16, padded to 768 cols
                means = mpool.tile([128, 768], dt.float16, tag="m")
                nc.scalar.activation(
                    means[:, 0:D_SPLIT],
                    ps_a[:],
                    mybir.ActivationFunctionType.Copy,
                    scale=recip_t[:, g : g + 1],
                )
                nc.scalar.activation(
                    means[:, D_SPLIT:D],
                    ps_b[:],
                    mybir.ActivationFunctionType.Copy,
                    scale=recip_t[:, g : g + 1],
                )
                if g < 2:
                    # first use of each double-buffered means tile; the pad
                    # columns are never written again, zeros persist
                    nc.vector.memset(means[:, D:768], 0.0)

                h = g % 2
                if h == 0:
                    mgt = tpool.tile([128, 6, 256], dt.float16, tag="mgt")
                for d in range(D_CHUNKS):
                    ps_t = ptpool.tile([128, 2, 128], dt.float16, tag="pt")
                    nc.tensor.transpose(
                        ps_t[:, 0, :],
                        means[:, d * 128 : (d + 1) * 128],
                        ident_t[:],
                    )
                    dst = mgt[:, d, h * 128 : h * 128 + 128]
                    if d % 2 == 0:
                        nc.vector.tensor_copy(dst, ps_t[:, 0, :])
                    else:
                        nc.scalar.copy(dst, ps_t[:, 0, :])
                if h == 1:
                    q = g // 2
                    pp = pppool.tile([128, 256], dt.float32, tag="pp")
                    for d in range(D_CHUNKS):
                        nc.tensor.matmul(
                            pp[:],
                            wt_t[:, d * 128 : (d + 1) * 128],
                            mgt[:, d, :],
                            start=(d == 0),
                            stop=(d == D_CHUNKS - 1),
                        )
                    if q % 4 == 0:
                        out_sb = opool.tile([C, 1024], dt.float32, tag="o")
                    nc.scalar.activation(
                        out_sb[:, (q % 4) * 256 : (q % 4 + 1) * 256],
                        pp[0:C, :],
                        mybir.ActivationFunctionType.Identity,
                        bias=bias_t[:],
                    )
                    if q % 4 == 3 or q == n_pairs - 1:
                        q0 = q - q % 4
                        nc.gpsimd.dma_start(
                            out=out_d[:, q0 * 256 : (q + 1) * 256],
                            in_=out_sb[:, 0 : (q % 4 + 1) * 256],
                        )
    return nc


def prepare(x, scope, rel_weight, bias, serial=False, dedup=True):
    split_multi_waits = _apply_walrus_workarounds()

    x = np.asarray(x, dtype=np.float32)
    scope_np = np.asarray(scope)
    rel_weight = np.asarray(rel_weight, dtype=np.float32)
    bias = np.asarray(bias, dtype=np.float32)
    n_bags = scope_np.shape[0] - 1

    cores, GB, GS, G, n_pairs = _preprocess(x, scope_np)
    nc = _build_program(GB, GS, G, n_pairs, serial=serial)
    if dedup:
        _dedup_ldweights(nc)
    split_multi_waits(nc)

    iota = np.tile(np.arange(128, dtype=np.float32), (128, 1))
    ident = np.eye(128, dtype=np.float16)
    wt = np.zeros((128, 768), dtype=np.float16)
    wpad = np.zeros((C, 768), dtype=np.float32)
    wpad[:, :D] = rel_weight
    for d in range(6):
        wt[:, d * 128 : d * 128 + C] = wpad[:, d * 128 : (d + 1) * 128].T
    bias_in = bias.reshape(C, 1).copy()

    in_maps = []
    for c in range(N_CORES):
        cd = cores[c]
        in_maps.append(
            {
                "x_big": cd["x_big"],
                "x_small": cd["x_small"],
                "seg_big": cd["seg_big"],
                "seg_small": cd["seg_small"],
                "recip": cd["recip"],
                "iota": iota,
                "ident": ident,
                "wt": wt,
                "bias": bias_in,
            }
        )

    def assemble(results):
        logits_t = np.empty((C, n_bags), dtype=np.float32)
        for c in range(N_CORES):
            out = results[c]["out"]  # [C, n_pairs*256]
            meta = cores[c]["meta"]
            for g in range(G):
                bags = meta[g]
                if len(bags) == 0:
                    continue
                col0 = 256 * (g // 2) + 128 * (g % 2)
                logits_t[:, bags] = out[:, col0 : col0 + len(bags)]
        return np.ascontiguousarray(logits_t.T)

    return dict(
        nc=nc, in_maps=in_maps, assemble=assemble, G=G, GB=GB, GS=GS,
        n_pairs=n_pairs,
    )


def kernel(x, scope, rel_weight, bias):
    from concourse.bass_utils import run_bass_kernel_spmd

    p = prepare(x, scope, rel_weight, bias)
    res = run_bass_kernel_spmd(p["nc"], p["in_maps"], list(range(N_CORES)))
    return p["assemble"](res.results)
